# revision 20
# baseline (speedup 1.0000x reference)
"""Multi-head attention (B=2, S=2048, D=1024, H=16) on 8 Trainium2 NeuronCores.

Sharding: core c handles (batch b=c//4, head-group g=c%4 of 4 heads) for ALL
2048 queries — head/tensor parallel, no collectives; the host sums the 4
partial output projections per batch.

v2 structure (vs the 229.9us baseline):
 - Scores use 2-way ROW TILING: per (qc, t, kk) slot, the two heads of a
   kt pair run as CONCURRENT K=64 matmuls at row positions 0 and 64
   (separate row-groups stream in parallel), halving score PE time and
   removing the Q zero-padding + its memsets.
 - Q/K projections evacuate straight into qpair/kt tiles (no half-row
   splits); their psums use the "op" tag ring so the "sc" ring stays a
   clean 2-deep score->exp pipeline. Proj/op emissions never interleave
   inside an out-proj psum pair's phase0..phase1 window (ring deadlock).
 - exp runs once per slot on [128, 1024] PSUM pairs (ACT is the pacing
   engine at ~1147ns/slot).
 - Batched DMA: one strided DMA per weight tensor and 4 column-block
   DMAs for x (instead of ~50 small transfers) so the first matmul can
   start at ~3.5us and HAM warms early.
 - PV keeps the [V|1] ones-column trick (M=65) for the softmax
   denominator; normalize multiplies straight out of PSUM.
 - Tail: the final (qc=3, t=1) normalize broadcasts 1/Z via a PE ones
   matmul and multiplies in 128-col chunks so the last out-proj
   matmuls start sooner; final evacs split ACT/DVE.
"""

import numpy as np
import ml_dtypes

import concourse.bass as bass
import concourse.mybir as mybir
import concourse.tile as tile
from concourse import bacc
from concourse.bass_utils import run_bass_kernel_spmd

BF16 = mybir.dt.bfloat16
F32 = mybir.dt.float32
AF = mybir.ActivationFunctionType

B, S, D = 2, 2048, 1024
H, HD = 16, 64
N_CORES = 8
G = 4              # head-groups per batch (cores per batch)
HL = H // G        # heads per core (4)
FL = HL * HD       # local projected features (256)
P = 128
DCH = D // P       # 8 contraction chunks
NKK = S // P       # 16 key chunks
QC = 512           # query block
NQC = S // QC      # 4
VW = HL * (HD + 1) + HD  # packed [V|1] width + 64 pad


def build_program():
    nc = bacc.Bacc("TRN2", target_bir_lowering=False, debug=False,
                   num_devices=N_CORES)

    xq_d = [nc.dram_tensor(f"xq{ch}", [P, DCH, QC], BF16,
                           kind="ExternalInput") for ch in range(NQC)]
    wqT = nc.dram_tensor("wqT", [P, 2, DCH, P], BF16, kind="ExternalInput")
    wkT = nc.dram_tensor("wkT", [P, 2, DCH, P], BF16, kind="ExternalInput")
    wvT = nc.dram_tensor("wvT", [P, DCH, FL], BF16, kind="ExternalInput")
    woT = nc.dram_tensor("woT", [P, 2, D], BF16, kind="ExternalInput")
    bqk = nc.dram_tensor("bqk", [P, 4], F32, kind="ExternalInput")
    out = nc.dram_tensor("out", [S, D], BF16, kind="ExternalOutput")

    with tile.TileContext(nc) as tc:
        _build(nc, tc, xq_d, wqT, wkT, wvT, woT, bqk, out)
    nc.compile()
    return nc


def _build(nc, tc, xq_d, wqT, wkT, wvT, woT, bqk, out):
    from contextlib import ExitStack

    ctx = ExitStack()
    consts = ctx.enter_context(tc.tile_pool(name="consts", bufs=1))
    bqk_sb = consts.tile([P, 4], F32, name="bqk_sb")
    ones_f = consts.tile([1, HD], F32, name="ones_f")
    nc.vector.memset(ones_f[:], 1.0)
    wup = consts.tile([P, P], BF16, name="wup")
    nc.vector.memset(wup[:], 0.0)

    # ---- batched input DMA (host pre-layouts everything [p, d, f]) ----
    xt_pool = ctx.enter_context(tc.tile_pool(name="xt", bufs=1))
    xq = [xt_pool.tile([P, DCH, QC], BF16, name=f"xq{ch}")
          for ch in range(NQC)]
    w_pool = ctx.enter_context(tc.tile_pool(name="w", bufs=1))
    wk_all = w_pool.tile([P, 2, DCH, P], BF16, name="wk_all")
    wq_all = w_pool.tile([P, 2, DCH, P], BF16, name="wq_all")
    wv_all = w_pool.tile([P, DCH, FL], BF16, name="wv_all")
    wo_all = w_pool.tile([P, 2, D], BF16, name="wo_all")

    # 3 parallel DMA queues, d-third granularity, in consumption order:
    # wk + x0 (first-matmul critical path), wq, wv, then the rest.
    qs = [nc.sync, nc.gpsimd, nc.scalar]
    parts = [slice(0, 3), slice(3, 6), slice(6, 8)]
    nc.sync.dma_start(bqk_sb[:], bqk.ap())
    for q, dq in zip(qs, parts):
        q.dma_start(wk_all[:, 0, dq, :], wkT.ap()[:, 0, dq, :])
    for q, dq in zip(qs, parts):
        q.dma_start(xq[0][:, dq, :], xq_d[0].ap()[:, dq, :])
    for q, dq in zip(qs, parts):
        q.dma_start(wq_all[:, 0, dq, :], wqT.ap()[:, 0, dq, :])
    for q, dq in zip(qs, parts):
        q.dma_start(wv_all[:, dq, :], wvT.ap()[:, dq, :])
    for ch in range(1, 4):
        for q, dq in zip(qs, parts):
            q.dma_start(xq[ch][:, dq, :], xq_d[ch].ap()[:, dq, :])
    for q, dq in zip(qs, parts):
        q.dma_start(wk_all[:, 1, dq, :], wkT.ap()[:, 1, dq, :])
    for q, dq in zip(qs, parts):
        q.dma_start(wq_all[:, 1, dq, :], wqT.ap()[:, 1, dq, :])
    nc.sync.dma_start(wo_all[:], woT.ap())

    # ---- persistent compute tiles ----
    kv_pool = ctx.enter_context(tc.tile_pool(name="kv", bufs=1))
    kt = [kv_pool.tile([P, S], BF16, name=f"kt{t}") for t in range(2)]
    qpair = [kv_pool.tile([P, S], BF16, name=f"qp{t}") for t in range(2)]
    vt = [kv_pool.tile([P, VW], BF16, name=f"vt{g}") for g in range(NKK)]
    for g in range(NKK):
        v3 = vt[g][:, 0:HL * (HD + 1)].rearrange("p (h c) -> p h c", c=HD + 1)
        nc.vector.memset(v3[:, :, HD:HD + 1], 1.0)
        nc.vector.memset(vt[g][:, HL * (HD + 1):VW], 0.0)
    attn_sb = [kv_pool.tile([P, S], BF16, name=f"asb{t}") for t in range(2)]

    small_pool = ctx.enter_context(tc.tile_pool(name="small", bufs=4))
    osb_pool = ctx.enter_context(tc.tile_pool(name="osb", bufs=4))

    # PSUM: sc 2x[128,1024]=4 banks, at 2x[65,512]=2, op 2x[128,512]=2.
    with tc.tile_pool(name="sc_ps", bufs=2, space="PSUM") as sc_ps, \
         tc.tile_pool(name="at_ps", bufs=2, space="PSUM") as at_ps, \
         tc.tile_pool(name="op_ps", bufs=2, space="PSUM") as op_ps, \
         tc.tile_pool(name="e_sb", bufs=10) as e_pool:

        # HAM warmup: ~12us of zero matmuls while the input DMAs land, so
        # the PE clock is at 8/8 when real work starts (throttle_active
        # was ~19us otherwise). Results are never read.
        wps = sc_ps.tile([P, 2 * QC], F32, name="wps", tag="sc")
        for i in range(100):
            nc.tensor.matmul(wps[:, 0:P], wup[:], wup[:],
                             start=(i == 0), stop=(i == 99))

        def proj_half(which, t, j, act=False):
            """K or Q projection for tile t, columns 512j:512(j+1)."""
            ps = op_ps.tile([P, QC], F32, name=f"ps{which}{t}_{j}", tag="op")
            w = wk_all if which == "k" else wq_all
            co = QC * j
            for d in range(DCH):
                nc.tensor.matmul(ps[:],
                                 w[:, t, d, :],
                                 xq[j][:, d, :],
                                 start=(d == 0), stop=(d == DCH - 1))
            dst = kt[t] if which == "k" else qpair[t]
            bcol = (2 + t) if which == "k" else t
            if act:
                nc.scalar.activation(dst[:, co:co + QC], ps[:], AF.Identity,
                                     bias=bqk_sb[:, bcol:bcol + 1])
            else:
                nc.vector.tensor_scalar_add(dst[:, co:co + QC], ps[:],
                                            bqk_sb[:, bcol:bcol + 1])

        def vproj_pair(pr):
            ps = op_ps.tile([P, 2 * FL], F32, name=f"psv{pr}", tag="op")
            for half in range(2):
                st = 2 * pr + half
                for d in range(DCH):
                    nc.tensor.matmul(ps[:, FL * half:FL * (half + 1)],
                                     xq[st // 4][:, d,
                                                 P * (st % 4):
                                                 P * (st % 4 + 1)],
                                     wv_all[:, d, :],
                                     start=(d == 0), stop=(d == DCH - 1))
            for half in range(2):
                st = 2 * pr + half
                v3 = vt[st][:, 0:HL * (HD + 1)].rearrange(
                    "p (h c) -> p h c", c=HD + 1)
                nc.vector.tensor_copy(
                    v3[:, :, 0:HD],
                    ps[:, FL * half:FL * (half + 1)].rearrange(
                        "p (h dd) -> p h dd", dd=HD))

        op_live = {}

        def run_op_half(st, phase, act_evac=False):
            if phase == 0:
                ops = [op_ps.tile([P, QC], F32, name=f"op{st}_{eb}", tag="op")
                       for eb in range(2)]
                op_live[st] = ops
                for eb in range(2):
                    nc.tensor.matmul(ops[eb][:],
                                     attn_sb[0][:, P * st:P * (st + 1)],
                                     wo_all[:, 0, QC * eb:QC * (eb + 1)],
                                     start=True, stop=False)
                return
            ops = op_live.pop(st)
            for eb in range(2):
                nc.tensor.matmul(ops[eb][:],
                                 attn_sb[1][:, P * st:P * (st + 1)],
                                 wo_all[:, 1, QC * eb:QC * (eb + 1)],
                                 start=False, stop=True)
            osb = osb_pool.tile([P, D], BF16, name=f"osb{st}", tag="osb")
            if act_evac:
                nc.scalar.activation(osb[:, 0:QC], ops[0][:], AF.Identity)
                nc.sync.dma_start(out.ap()[P * st:P * (st + 1), 0:QC],
                                  osb[:, 0:QC])
                nc.vector.tensor_copy(osb[:, QC:2 * QC], ops[1][:])
                nc.sync.dma_start(out.ap()[P * st:P * (st + 1), QC:2 * QC],
                                  osb[:, QC:2 * QC])
            else:
                nc.vector.tensor_copy(osb[:, 0:QC], ops[0][:])
                nc.vector.tensor_copy(osb[:, QC:2 * QC], ops[1][:])
                nc.sync.dma_start(out.ap()[P * st:P * (st + 1), :], osb[:])

        def normalize_evac(qc, t, par, at, fine=False):
            """One [65,512] copy evacuates the whole at-psum tile (frees
            the ring slot in a single op)."""
            atz = small_pool.tile([HD + 1, QC], F32,
                                  name=f"atz{qc}_{t}_{par}", tag="atz")
            if fine and par == 1:
                nc.scalar.activation(atz[:], at[:], AF.Identity)
            else:
                nc.vector.tensor_copy(atz[:], at[:])
            return atz

        def normalize(qc, t, par, atz, fine=False):
            """attn_sb[t][64par:64par+64, qc] = atz[0:64]/atz[64]."""
            zr = small_pool.tile([1, QC], F32, name=f"zr{qc}_{t}_{par}",
                                 tag="zr")
            if fine and par == 0:
                nc.scalar.activation(zr[:], atz[HD:HD + 1, :], AF.Identity)
            else:
                nc.vector.tensor_copy(zr[:], atz[HD:HD + 1, :])
            rz = small_pool.tile([1, QC], F32, name=f"rz{qc}_{t}_{par}",
                                 tag="rz")
            nc.vector.reciprocal_approx_fast(rz[:], zr[:])
            if fine:
                # PE broadcast via ones (PE is free at the tail) + chunked
                # multiplies so the last out-proj can start per 128 cols.
                rzp = op_ps.tile([HD, QC], F32, name=f"rzp{qc}_{t}_{par}",
                                 tag="op")
                nc.tensor.matmul(rzp[:], ones_f[:], rz[:],
                                 start=True, stop=True)
                rzsb = small_pool.tile([HD, QC], F32,
                                       name=f"rzsb{qc}_{t}_{par}", tag="rzb")
                nc.vector.tensor_copy(rzsb[:], rzp[:])
                for ch in range(4):
                    cs = slice(P * ch, P * (ch + 1))
                    nc.vector.tensor_mul(
                        attn_sb[t][HD * par:HD * (par + 1),
                                   QC * qc + P * ch:QC * qc + P * (ch + 1)],
                        atz[0:HD, cs], rzsb[:, cs])
            else:
                rzb = small_pool.tile([HD, QC], F32,
                                      name=f"rzb{qc}_{t}_{par}", tag="rzb")
                nc.gpsimd.partition_broadcast(rzb[:], rz[:])
                mul_queue.append((qc, t, par, atz, rzb))

        # ---- slot schedule ----
        # slot s = qc*32 + t*16 + kk ; each slot: 2 concurrent score MMs
        # (row-tiled par 0/1) -> exp[128,1024] -> (pend) 2 PV MMs.
        slots = [(qc, t, kk) for qc in range(NQC) for t in range(2)
                 for kk in range(NKK)]

        # proj work queue: (deadline_slot, kind, (t, j) or g)
        projq = []
        for t in range(2):
            for j in range(NQC):
                if (t, j) != (0, 0):
                    projq.append((t * 16 + 4 * j, "k", (t, j)))
                    projq.append((max(0, j * 32 + t * 16 - 6), "q", (t, j)))
        for g in range(1, DCH):
            projq.append((2 * g, "v", g))
        projq.sort(key=lambda x: x[0])

        mul_queue = []

        def emit_muls():
            while mul_queue:
                qc, t, par, atz, rzb = mul_queue.pop(0)
                nc.vector.tensor_mul(
                    attn_sb[t][HD * par:HD * (par + 1),
                               QC * qc:QC * (qc + 1)],
                    atz[0:HD, :], rzb[:])

        pend = []            # (qc, t, kk, par, slot, e_tile, col_off)
        at_tiles = {}
        vready = {}
        op_queue = []
        norm_queue = []

        def flush_norms(s):
            evacs = [(e, normalize_evac(*e[:3], e[3], fine=e[4]))
                     for e in norm_queue]
            del norm_queue[:]
            for (qc, t, par, _, fine), atz in evacs:
                normalize(qc, t, par, atz, fine=fine)
                if par == 1 and t == 1:
                    op_queue.extend((st, ph, s + 3)
                                    for st in range(4 * qc, 4 * qc + 4)
                                    for ph in range(2))

        def drain_one(s, force=False):
            if not pend:
                return False
            qc, t, kk, par, g, et, off = pend[0]
            if not force:
                if g >= s:
                    return False
                if vready.get(kk // 2, 10 ** 9) > s - 2:
                    return False
            pend.pop(0)
            key = (qc, t, par)
            if key not in at_tiles:
                at_tiles[key] = at_ps.tile([HD + 1, QC], F32,
                                           name=f"at{qc}_{t}_{par}", tag="at")
            h = 2 * t + par
            nc.tensor.matmul(at_tiles[key][:], vt[kk][:, 65 * h:65 * h + 65],
                             et[:, off:off + QC],
                             start=(kk == 0), stop=(kk == NKK - 1))
            if kk == NKK - 1:
                last = (qc == NQC - 1 and t == 1)
                norm_queue.append((qc, t, par, at_tiles.pop(key), last))
            return True

        # preamble: minimum before the first score slot
        proj_half("k", 0, 0, act=True)
        proj_half("q", 0, 0, act=True)
        projq.insert(0, (0, "v", 0))

        for s, (qc, t, kk) in enumerate(slots):
            while projq and projq[0][0] <= s + 3 and not op_live:
                _, kind, arg = projq.pop(0)
                if kind == "v":
                    vready[arg] = s
                    vproj_pair(arg)
                else:
                    proj_half(kind, *arg)
            sc = sc_ps.tile([P, 2 * QC], F32, name=f"sc{s}", tag="sc")
            for par in range(2):
                nc.tensor.matmul(sc[:, QC * par:QC * (par + 1)],
                                 kt[t][HD * par:HD * (par + 1),
                                       P * kk:P * (kk + 1)],
                                 qpair[t][HD * par:HD * (par + 1),
                                          QC * qc:QC * (qc + 1)],
                                 start=True, stop=True)
            e = e_pool.tile([P, 2 * QC], BF16, name=f"e{s}", tag="e")
            nc.scalar.activation(e[:], sc[:], AF.Exp)
            for par in range(2):
                pend.append((qc, t, kk, par, s, e, QC * par))
            drained = 0
            while drained < 3 and drain_one(s):
                drained += 1
            emit_muls()
            flush_norms(s)
            if s % 2 == 1 and op_queue and op_queue[0][2] <= s:
                st, ph, _ = op_queue.pop(0)
                run_op_half(st, ph)
        while pend:
            drain_one(10 ** 9, force=True)
        emit_muls()
        flush_norms(10 ** 9)
        emit_muls()
        nq = len(op_queue)
        for i in range(nq):
            st, ph, _ = op_queue.pop(0)
            run_op_half(st, ph, act_evac=(i >= nq - 4))

    ctx.close()


_CACHE = {}


def _get_program():
    if "nc" not in _CACHE:
        _CACHE["nc"] = build_program()
    return _CACHE["nc"]


def prep_inputs(input_tensor, qkv_weight, qkv_bias, out_weight, out_bias):
    """Host-side shard + transpose + cast. Returns in_maps for 8 cores."""
    x = np.asarray(input_tensor, np.float32)
    wqkv = np.asarray(qkv_weight, np.float32).copy()
    bqkv = np.asarray(qkv_bias, np.float32).copy()
    wout = np.asarray(out_weight, np.float32)
    scale = 1.0 / np.sqrt(np.float32(HD))
    wqkv[:D] *= scale
    bqkv[:D] *= scale
    bf = ml_dtypes.bfloat16

    def pdf(wT):
        # [D, F] -> [p, d, F] with D = d*128 + p
        return np.ascontiguousarray(
            wT.reshape(DCH, P, -1).transpose(1, 0, 2)).astype(bf)

    woutT = np.ascontiguousarray(wout.T)
    # x[b].T is [D, S]; slabs [p, d, 512] per column block
    xqb = []
    for b in range(B):
        xt = x[b].T.reshape(DCH, P, S).transpose(1, 0, 2)
        xqb.append([np.ascontiguousarray(xt[:, :, QC * ch:QC * (ch + 1)])
                    .astype(bf) for ch in range(NQC)])
    in_maps = []
    for c in range(N_CORES):
        b, g = c // G, c % G
        lo = FL * g
        def pdf_t(wT):
            # [D, F=256] -> [p, t, d, 128]: F = t*128 + fp, D = d*128 + p
            a = wT.reshape(DCH, P, 2, P)          # [d, p, t, fp]
            return np.ascontiguousarray(
                a.transpose(1, 2, 0, 3)).astype(bf)

        wqT = pdf_t(wqkv[lo:lo + FL, :].T)
        wkT = pdf_t(wqkv[D + lo:D + lo + FL, :].T)
        wvT = pdf(wqkv[2 * D + lo:2 * D + lo + FL, :].T)
        woTg = np.ascontiguousarray(
            woutT[lo:lo + FL, :].reshape(2, P, D).transpose(1, 0, 2)
        ).astype(bf)
        bq = bqkv[lo:lo + FL].reshape(2, P).T
        bk = bqkv[D + lo:D + lo + FL].reshape(2, P).T
        bqk = np.ascontiguousarray(
            np.concatenate([bq, bk], 1)).astype(np.float32)
        im = {"wqT": wqT, "wkT": wkT, "wvT": wvT,
              "woT": woTg, "bqk": bqk}
        for ch in range(NQC):
            im[f"xq{ch}"] = xqb[b][ch]
        in_maps.append(im)
    return in_maps


def assemble(outs, qkv_bias, out_weight, out_bias):
    """Sum the per-core partials and add the (V-bias-folded) output bias."""
    bqkv = np.asarray(qkv_bias, np.float32)
    wout = np.asarray(out_weight, np.float32)
    bout_eff = np.asarray(out_bias, np.float32) + wout @ bqkv[2 * D:]
    full = np.empty((B, S, D), np.float32)
    for b in range(B):
        acc = bout_eff[None, :].astype(np.float32).repeat(S, 0)
        for g in range(G):
            acc += np.asarray(outs[b * G + g], np.float32)
        full[b] = acc
    return full


def kernel(input_tensor, qkv_weight, qkv_bias, out_weight, out_bias,
           **run_kwargs):
    nc = _get_program()
    in_maps = prep_inputs(input_tensor, qkv_weight, qkv_bias, out_weight,
                          out_bias)
    res = run_bass_kernel_spmd(nc, in_maps, core_ids=list(range(N_CORES)),
                               **run_kwargs)
    full = assemble([res.results[c]["out"] for c in range(N_CORES)],
                    qkv_bias, out_weight, out_bias)
    if run_kwargs:
        kernel.last_results = res
    return full


# revision 26
# speedup vs baseline: 1.0097x; 1.0097x over previous
"""Multi-head attention (B=2, S=2048, D=1024, H=16) on 8 Trainium2 NeuronCores.

Sharding: core c handles (batch b=c//4, head-group g=c%4 of 4 heads) for ALL
2048 queries — head/tensor parallel, no collectives; the host sums the 4
partial output projections per batch (assemble()).

Structure (~216us vs the 230us query-parallel baseline; PE-bound at the
bf16 roofline; ACT exp [16.8M elem/core] runs just under the PE):
 - Scores use 2-way ROW TILING: per (qc, t, kk) slot, the two heads of a
   kt pair run as CONCURRENT K=64 matmuls at row positions 0 and 64
   (~320ns/pair vs 427ns serial; removes Q zero-padding + memsets).
 - One exp per slot on a [128,1024] PSUM pair; sc ring is a clean
   2-deep score->exp pipeline; K/Q/V projections use the "op" psum ring
   (never interleaved inside an out-proj phase0..phase1 window -- ring
   deadlock). Host pre-lays x/w in [partition, chunk, col] form so all
   DMAs are contiguous, split across the 3 DMA-capable queues
   (sync/gpsimd/scalar) in consumption-deadline order.
 - PV keeps the [V|1] ones-column (M=65) softmax-denominator trick; a
   single [65,512] copy evacuates each at-psum (frees the 2-deep ring);
   1/Z via fast DVE reciprocal + gpsimd partition_broadcast.
 - ~56 zero matmuls warm the HAM clock gate (4/8 -> 8/8) during the
   input-DMA wait; the final (qc=3,t=1) normalize broadcasts 1/Z via a
   PE ones-matmul and multiplies in 128-col chunks so the tail
   out-proj starts per-block; final evacs split ACT/DVE, final output
   DMA issued per 512-col half.

fp8/DoubleRow was evaluated and rejected: e4m3 quantization of any
matmul operand adds ~2-15% output error (tolerance is 2e-2; bf16
everywhere measures 4.8e-3).
"""

import numpy as np
import ml_dtypes

import concourse.bass as bass
import concourse.mybir as mybir
import concourse.tile as tile
from concourse import bacc
from concourse.bass_utils import run_bass_kernel_spmd

BF16 = mybir.dt.bfloat16
F32 = mybir.dt.float32
AF = mybir.ActivationFunctionType

B, S, D = 2, 2048, 1024
H, HD = 16, 64
N_CORES = 8
G = 4              # head-groups per batch (cores per batch)
HL = H // G        # heads per core (4)
FL = HL * HD       # local projected features (256)
P = 128
DCH = D // P       # 8 contraction chunks
NKK = S // P       # 16 key chunks
QC = 512           # query block
NQC = S // QC      # 4
VW = HL * (HD + 1) + HD  # packed [V|1] width + 64 pad


def build_program():
    nc = bacc.Bacc("TRN2", target_bir_lowering=False, debug=False,
                   num_devices=N_CORES)

    xq_d = [nc.dram_tensor(f"xq{ch}", [P, DCH, QC], BF16,
                           kind="ExternalInput") for ch in range(NQC)]
    wqT = nc.dram_tensor("wqT", [P, 2, DCH, P], BF16, kind="ExternalInput")
    wkT = nc.dram_tensor("wkT", [P, 2, DCH, P], BF16, kind="ExternalInput")
    wvT = nc.dram_tensor("wvT", [P, DCH, FL], BF16, kind="ExternalInput")
    woT = nc.dram_tensor("woT", [P, 2, D], BF16, kind="ExternalInput")
    bqk = nc.dram_tensor("bqk", [P, 4], F32, kind="ExternalInput")
    out = nc.dram_tensor("out", [S, D], BF16, kind="ExternalOutput")

    with tile.TileContext(nc) as tc:
        _build(nc, tc, xq_d, wqT, wkT, wvT, woT, bqk, out)
    nc.compile()
    return nc


def _build(nc, tc, xq_d, wqT, wkT, wvT, woT, bqk, out):
    from contextlib import ExitStack

    ctx = ExitStack()
    consts = ctx.enter_context(tc.tile_pool(name="consts", bufs=1))
    bqk_sb = consts.tile([P, 4], F32, name="bqk_sb")
    ones_f = consts.tile([1, HD], F32, name="ones_f")
    nc.vector.memset(ones_f[:], 1.0)
    wup = consts.tile([P, P], BF16, name="wup")
    nc.vector.memset(wup[:], 0.0)

    # ---- batched input DMA (host pre-layouts everything [p, d, f]) ----
    xt_pool = ctx.enter_context(tc.tile_pool(name="xt", bufs=1))
    xq = [xt_pool.tile([P, DCH, QC], BF16, name=f"xq{ch}")
          for ch in range(NQC)]
    w_pool = ctx.enter_context(tc.tile_pool(name="w", bufs=1))
    wk_all = w_pool.tile([P, 2, DCH, P], BF16, name="wk_all")
    wq_all = w_pool.tile([P, 2, DCH, P], BF16, name="wq_all")
    wv_all = w_pool.tile([P, DCH, FL], BF16, name="wv_all")
    wo_all = w_pool.tile([P, 2, D], BF16, name="wo_all")

    # 3 parallel DMA queues, d-third granularity, in consumption order:
    # wk + x0 (first-matmul critical path), wq, wv, then the rest.
    qs = [nc.sync, nc.gpsimd, nc.scalar]
    parts = [slice(0, 3), slice(3, 6), slice(6, 8)]
    nc.sync.dma_start(bqk_sb[:], bqk.ap())
    for q, dq in zip(qs, parts):
        q.dma_start(wk_all[:, 0, dq, :], wkT.ap()[:, 0, dq, :])
    for q, dq in zip(qs, parts):
        q.dma_start(xq[0][:, dq, :], xq_d[0].ap()[:, dq, :])
    for q, dq in zip(qs, parts):
        q.dma_start(wq_all[:, 0, dq, :], wqT.ap()[:, 0, dq, :])
    for q, dq in zip(qs, parts):
        q.dma_start(wv_all[:, dq, :], wvT.ap()[:, dq, :])
    for ch in range(1, 4):
        for q, dq in zip(qs, parts):
            q.dma_start(xq[ch][:, dq, :], xq_d[ch].ap()[:, dq, :])
    for q, dq in zip(qs, parts):
        q.dma_start(wk_all[:, 1, dq, :], wkT.ap()[:, 1, dq, :])
    for q, dq in zip(qs, parts):
        q.dma_start(wq_all[:, 1, dq, :], wqT.ap()[:, 1, dq, :])
    nc.sync.dma_start(wo_all[:], woT.ap())

    # ---- persistent compute tiles ----
    kv_pool = ctx.enter_context(tc.tile_pool(name="kv", bufs=1))
    kt = [kv_pool.tile([P, S], BF16, name=f"kt{t}") for t in range(2)]
    qpair = [kv_pool.tile([P, S], BF16, name=f"qp{t}") for t in range(2)]
    vt = [kv_pool.tile([P, VW], BF16, name=f"vt{g}") for g in range(NKK)]
    for g in range(NKK):
        v3 = vt[g][:, 0:HL * (HD + 1)].rearrange("p (h c) -> p h c", c=HD + 1)
        nc.vector.memset(v3[:, :, HD:HD + 1], 1.0)
        nc.vector.memset(vt[g][:, HL * (HD + 1):VW], 0.0)
    attn_sb = [kv_pool.tile([P, S], BF16, name=f"asb{t}") for t in range(2)]

    small_pool = ctx.enter_context(tc.tile_pool(name="small", bufs=4))
    osb_pool = ctx.enter_context(tc.tile_pool(name="osb", bufs=4))

    # PSUM: sc 2x[128,1024]=4 banks, at 2x[65,512]=2, op 2x[128,512]=2.
    with tc.tile_pool(name="sc_ps", bufs=2, space="PSUM") as sc_ps, \
         tc.tile_pool(name="at_ps", bufs=2, space="PSUM") as at_ps, \
         tc.tile_pool(name="op_ps", bufs=2, space="PSUM") as op_ps, \
         tc.tile_pool(name="e_sb", bufs=10) as e_pool:

        # HAM warmup: ~8us of zero matmuls while the input DMAs land, so
        # the PE clock is at 8/8 when real work starts. Results unread.
        wps = sc_ps.tile([P, 2 * QC], F32, name="wps", tag="sc")
        for i in range(56):
            nc.tensor.matmul(wps[:, 0:P], wup[:], wup[:],
                             start=(i == 0), stop=(i == 55))


        def proj_half(which, t, j, act=False):
            """K or Q projection for tile t, columns 512j:512(j+1)."""
            ps = op_ps.tile([P, QC], F32, name=f"ps{which}{t}_{j}", tag="op")
            w = wk_all if which == "k" else wq_all
            co = QC * j
            for d in range(DCH):
                nc.tensor.matmul(ps[:],
                                 w[:, t, d, :],
                                 xq[j][:, d, :],
                                 start=(d == 0), stop=(d == DCH - 1))
            dst = kt[t] if which == "k" else qpair[t]
            bcol = (2 + t) if which == "k" else t
            if act:
                nc.scalar.activation(dst[:, co:co + QC], ps[:], AF.Identity,
                                     bias=bqk_sb[:, bcol:bcol + 1])
            else:
                nc.vector.tensor_scalar_add(dst[:, co:co + QC], ps[:],
                                            bqk_sb[:, bcol:bcol + 1])

        def vproj_pair(pr):
            ps = op_ps.tile([P, 2 * FL], F32, name=f"psv{pr}", tag="op")
            for half in range(2):
                st = 2 * pr + half
                for d in range(DCH):
                    nc.tensor.matmul(ps[:, FL * half:FL * (half + 1)],
                                     xq[st // 4][:, d,
                                                 P * (st % 4):
                                                 P * (st % 4 + 1)],
                                     wv_all[:, d, :],
                                     start=(d == 0), stop=(d == DCH - 1))
            for half in range(2):
                st = 2 * pr + half
                v3 = vt[st][:, 0:HL * (HD + 1)].rearrange(
                    "p (h c) -> p h c", c=HD + 1)
                nc.vector.tensor_copy(
                    v3[:, :, 0:HD],
                    ps[:, FL * half:FL * (half + 1)].rearrange(
                        "p (h dd) -> p h dd", dd=HD))

        op_live = {}

        def run_op_half(st, phase, act_evac=False):
            if phase == 0:
                ops = [op_ps.tile([P, QC], F32, name=f"op{st}_{eb}", tag="op")
                       for eb in range(2)]
                op_live[st] = ops
                for eb in range(2):
                    nc.tensor.matmul(ops[eb][:],
                                     attn_sb[0][:, P * st:P * (st + 1)],
                                     wo_all[:, 0, QC * eb:QC * (eb + 1)],
                                     start=True, stop=False)
                return
            ops = op_live.pop(st)
            for eb in range(2):
                nc.tensor.matmul(ops[eb][:],
                                 attn_sb[1][:, P * st:P * (st + 1)],
                                 wo_all[:, 1, QC * eb:QC * (eb + 1)],
                                 start=False, stop=True)
            osb = osb_pool.tile([P, D], BF16, name=f"osb{st}", tag="osb")
            if act_evac:
                nc.scalar.activation(osb[:, 0:QC], ops[0][:], AF.Identity)
                nc.sync.dma_start(out.ap()[P * st:P * (st + 1), 0:QC],
                                  osb[:, 0:QC])
                nc.vector.tensor_copy(osb[:, QC:2 * QC], ops[1][:])
                nc.sync.dma_start(out.ap()[P * st:P * (st + 1), QC:2 * QC],
                                  osb[:, QC:2 * QC])
            else:
                nc.vector.tensor_copy(osb[:, 0:QC], ops[0][:])
                nc.vector.tensor_copy(osb[:, QC:2 * QC], ops[1][:])
                nc.sync.dma_start(out.ap()[P * st:P * (st + 1), :], osb[:])

        def normalize_evac(qc, t, par, at, fine=False):
            """One [65,512] copy evacuates the whole at-psum tile (frees
            the ring slot in a single op)."""
            atz = small_pool.tile([HD + 1, QC], F32,
                                  name=f"atz{qc}_{t}_{par}", tag="atz")
            if fine and par == 1:
                nc.scalar.activation(atz[:], at[:], AF.Identity)
            else:
                nc.vector.tensor_copy(atz[:], at[:])
            return atz

        def normalize(qc, t, par, atz, fine=False):
            """attn_sb[t][64par:64par+64, qc] = atz[0:64]/atz[64]."""
            zr = small_pool.tile([1, QC], F32, name=f"zr{qc}_{t}_{par}",
                                 tag="zr")
            if fine and par == 0:
                nc.scalar.activation(zr[:], atz[HD:HD + 1, :], AF.Identity)
            else:
                nc.vector.tensor_copy(zr[:], atz[HD:HD + 1, :])
            rz = small_pool.tile([1, QC], F32, name=f"rz{qc}_{t}_{par}",
                                 tag="rz")
            nc.vector.reciprocal_approx_fast(rz[:], zr[:])
            if fine:
                # PE broadcast via ones (PE is free at the tail) + chunked
                # multiplies so the last out-proj can start per 128 cols.
                rzp = op_ps.tile([HD, QC], F32, name=f"rzp{qc}_{t}_{par}",
                                 tag="op")
                nc.tensor.matmul(rzp[:], ones_f[:], rz[:],
                                 start=True, stop=True)
                rzsb = small_pool.tile([HD, QC], F32,
                                       name=f"rzsb{qc}_{t}_{par}", tag="rzb")
                nc.vector.tensor_copy(rzsb[:], rzp[:])
                for ch in range(4):
                    cs = slice(P * ch, P * (ch + 1))
                    nc.vector.tensor_mul(
                        attn_sb[t][HD * par:HD * (par + 1),
                                   QC * qc + P * ch:QC * qc + P * (ch + 1)],
                        atz[0:HD, cs], rzsb[:, cs])
            else:
                rzb = small_pool.tile([HD, QC], F32,
                                      name=f"rzb{qc}_{t}_{par}", tag="rzb")
                nc.gpsimd.partition_broadcast(rzb[:], rz[:])
                nc.vector.tensor_mul(
                    attn_sb[t][HD * par:HD * (par + 1),
                               QC * qc:QC * (qc + 1)],
                    atz[0:HD, :], rzb[:])

        # ---- slot schedule ----
        # slot s = qc*32 + t*16 + kk ; each slot: 2 concurrent score MMs
        # (row-tiled par 0/1) -> exp[128,1024] -> (pend) 2 PV MMs.
        slots = [(qc, t, kk) for qc in range(NQC) for t in range(2)
                 for kk in range(NKK)]

        # proj work queue: (deadline_slot, kind, (t, j) or g)
        projq = []
        for t in range(2):
            for j in range(NQC):
                if (t, j) != (0, 0):
                    projq.append((t * 16 + 4 * j, "k", (t, j)))
                    projq.append((j * 32 + t * 16, "q", (t, j)))
        for g in range(1, DCH):
            projq.append((2 * g, "v", g))
        projq.sort(key=lambda x: x[0])

        pend = []            # (qc, t, kk, par, slot, e_tile, col_off)
        at_tiles = {}
        vready = {}
        op_queue = []
        norm_queue = []

        def flush_norms(s):
            evacs = [(e, normalize_evac(*e[:3], e[3], fine=e[4]))
                     for e in norm_queue]
            del norm_queue[:]
            for (qc, t, par, _, fine), atz in evacs:
                normalize(qc, t, par, atz, fine=fine)
                if par == 1 and t == 1:
                    op_queue.extend((st, ph, s + 2)
                                    for st in range(4 * qc, 4 * qc + 4)
                                    for ph in range(2))

        def drain_one(s, force=False):
            if not pend:
                return False
            qc, t, kk, par, g, et, off = pend[0]
            if not force:
                if g >= s:
                    return False
                if vready.get(kk // 2, 10 ** 9) > s - 2:
                    return False
            pend.pop(0)
            key = (qc, t, par)
            if key not in at_tiles:
                at_tiles[key] = at_ps.tile([HD + 1, QC], F32,
                                           name=f"at{qc}_{t}_{par}", tag="at")
            h = 2 * t + par
            nc.tensor.matmul(at_tiles[key][:], vt[kk][:, 65 * h:65 * h + 65],
                             et[:, off:off + QC],
                             start=(kk == 0), stop=(kk == NKK - 1))
            if kk == NKK - 1:
                last = (qc == NQC - 1 and t == 1)
                norm_queue.append((qc, t, par, at_tiles.pop(key), last))
            return True

        # preamble: minimum before the first score slot
        proj_half("k", 0, 0, act=True)
        proj_half("q", 0, 0, act=True)
        projq.insert(0, (0, "v", 0))

        for s, (qc, t, kk) in enumerate(slots):
            while projq and projq[0][0] <= s + 3 and not op_live:
                _, kind, arg = projq.pop(0)
                if kind == "v":
                    vready[arg] = s
                    vproj_pair(arg)
                else:
                    proj_half(kind, *arg)
            sc = sc_ps.tile([P, 2 * QC], F32, name=f"sc{s}", tag="sc")
            for par in range(2):
                nc.tensor.matmul(sc[:, QC * par:QC * (par + 1)],
                                 kt[t][HD * par:HD * (par + 1),
                                       P * kk:P * (kk + 1)],
                                 qpair[t][HD * par:HD * (par + 1),
                                          QC * qc:QC * (qc + 1)],
                                 start=True, stop=True)
            e = e_pool.tile([P, 2 * QC], BF16, name=f"e{s}", tag="e")
            nc.scalar.activation(e[:], sc[:], AF.Exp)
            for par in range(2):
                pend.append((qc, t, kk, par, s, e, QC * par))
            drained = 0
            while drained < 3 and drain_one(s):
                drained += 1
            flush_norms(s)
            if s % 2 == 1 and op_queue and op_queue[0][2] <= s:
                st, ph, _ = op_queue.pop(0)
                run_op_half(st, ph)
        while pend:
            drain_one(10 ** 9, force=True)
        flush_norms(10 ** 9)
        nq = len(op_queue)
        for i in range(nq):
            st, ph, _ = op_queue.pop(0)
            run_op_half(st, ph, act_evac=(i >= nq - 4))

    ctx.close()


_CACHE = {}


def _get_program():
    if "nc" not in _CACHE:
        _CACHE["nc"] = build_program()
    return _CACHE["nc"]


def prep_inputs(input_tensor, qkv_weight, qkv_bias, out_weight, out_bias):
    """Host-side shard + transpose + cast. Returns in_maps for 8 cores."""
    x = np.asarray(input_tensor, np.float32)
    wqkv = np.asarray(qkv_weight, np.float32).copy()
    bqkv = np.asarray(qkv_bias, np.float32).copy()
    wout = np.asarray(out_weight, np.float32)
    scale = 1.0 / np.sqrt(np.float32(HD))
    wqkv[:D] *= scale
    bqkv[:D] *= scale
    bf = ml_dtypes.bfloat16

    def pdf(wT):
        # [D, F] -> [p, d, F] with D = d*128 + p
        return np.ascontiguousarray(
            wT.reshape(DCH, P, -1).transpose(1, 0, 2)).astype(bf)

    woutT = np.ascontiguousarray(wout.T)
    # x[b].T is [D, S]; slabs [p, d, 512] per column block
    xqb = []
    for b in range(B):
        xt = x[b].T.reshape(DCH, P, S).transpose(1, 0, 2)
        xqb.append([np.ascontiguousarray(xt[:, :, QC * ch:QC * (ch + 1)])
                    .astype(bf) for ch in range(NQC)])
    in_maps = []
    for c in range(N_CORES):
        b, g = c // G, c % G
        lo = FL * g
        def pdf_t(wT):
            # [D, F=256] -> [p, t, d, 128]: F = t*128 + fp, D = d*128 + p
            a = wT.reshape(DCH, P, 2, P)          # [d, p, t, fp]
            return np.ascontiguousarray(
                a.transpose(1, 2, 0, 3)).astype(bf)

        wqT = pdf_t(wqkv[lo:lo + FL, :].T)
        wkT = pdf_t(wqkv[D + lo:D + lo + FL, :].T)
        wvT = pdf(wqkv[2 * D + lo:2 * D + lo + FL, :].T)
        woTg = np.ascontiguousarray(
            woutT[lo:lo + FL, :].reshape(2, P, D).transpose(1, 0, 2)
        ).astype(bf)
        bq = bqkv[lo:lo + FL].reshape(2, P).T
        bk = bqkv[D + lo:D + lo + FL].reshape(2, P).T
        bqk = np.ascontiguousarray(
            np.concatenate([bq, bk], 1)).astype(np.float32)
        im = {"wqT": wqT, "wkT": wkT, "wvT": wvT,
              "woT": woTg, "bqk": bqk}
        for ch in range(NQC):
            im[f"xq{ch}"] = xqb[b][ch]
        in_maps.append(im)
    return in_maps


def assemble(outs, qkv_bias, out_weight, out_bias):
    """Sum the per-core partials and add the (V-bias-folded) output bias."""
    bqkv = np.asarray(qkv_bias, np.float32)
    wout = np.asarray(out_weight, np.float32)
    bout_eff = np.asarray(out_bias, np.float32) + wout @ bqkv[2 * D:]
    full = np.empty((B, S, D), np.float32)
    for b in range(B):
        acc = bout_eff[None, :].astype(np.float32).repeat(S, 0)
        for g in range(G):
            acc += np.asarray(outs[b * G + g], np.float32)
        full[b] = acc
    return full


def kernel(input_tensor, qkv_weight, qkv_bias, out_weight, out_bias,
           **run_kwargs):
    nc = _get_program()
    in_maps = prep_inputs(input_tensor, qkv_weight, qkv_bias, out_weight,
                          out_bias)
    res = run_bass_kernel_spmd(nc, in_maps, core_ids=list(range(N_CORES)),
                               **run_kwargs)
    full = assemble([res.results[c]["out"] for c in range(N_CORES)],
                    qkv_bias, out_weight, out_bias)
    if run_kwargs:
        kernel.last_results = res
    return full


# revision 39
# speedup vs baseline: 1.2535x; 1.2414x over previous
"""Multi-head attention (B=2, S=2048, D=1024, H=16) on 8 Trainium2 NeuronCores.

Sharding: core c handles (batch b=c//4, head-group g=c%4 of 4 heads) for ALL
2048 queries — head/tensor parallel, no collectives; the host sums the 4
partial output projections per batch (assemble()).

Structure (~216us vs the 230us query-parallel baseline; PE-bound at the
bf16 roofline; ACT exp [16.8M elem/core] runs just under the PE):
 - Scores use 2-way ROW TILING: per (qc, t, kk) slot, the two heads of a
   kt pair run as CONCURRENT K=64 matmuls at row positions 0 and 64
   (~320ns/pair vs 427ns serial; removes Q zero-padding + memsets).
 - One exp per slot on a [128,1024] PSUM pair; sc ring is a clean
   2-deep score->exp pipeline; K/Q/V projections use the "op" psum ring
   (never interleaved inside an out-proj phase0..phase1 window -- ring
   deadlock). Host pre-lays x/w in [partition, chunk, col] form so all
   DMAs are contiguous, split across the 3 DMA-capable queues
   (sync/gpsimd/scalar) in consumption-deadline order.
 - PV keeps the [V|1] ones-column (M=65) softmax-denominator trick; a
   single [65,512] copy evacuates each at-psum (frees the 2-deep ring);
   1/Z via fast DVE reciprocal + gpsimd partition_broadcast.
 - ~56 zero matmuls warm the HAM clock gate (4/8 -> 8/8) during the
   input-DMA wait; the final (qc=3,t=1) normalize broadcasts 1/Z via a
   PE ones-matmul and multiplies in 128-col chunks so the tail
   out-proj starts per-block; final evacs split ACT/DVE, final output
   DMA issued per 512-col half.

fp8/DoubleRow was evaluated and rejected: e4m3 quantization of any
matmul operand adds ~2-15% output error (tolerance is 2e-2; bf16
everywhere measures 4.8e-3).
"""

import numpy as np
import ml_dtypes

import concourse.bass as bass
import concourse.mybir as mybir
import concourse.tile as tile
from concourse import bacc
from concourse.bass_utils import run_bass_kernel_spmd

BF16 = mybir.dt.bfloat16
F32 = mybir.dt.float32
AF = mybir.ActivationFunctionType

B, S, D = 2, 2048, 1024
H, HD = 16, 64
N_CORES = 8
G = 4              # head-groups per batch (cores per batch)
HL = H // G        # heads per core (4)
FL = HL * HD       # local projected features (256)
P = 128
DCH = D // P       # 8 contraction chunks
NKK = S // P       # 16 key chunks
QC = 512           # query block
NQC = S // QC      # 4
VW = HL * (HD + 1) + HD  # packed [V|1] width + 64 pad


def build_program():
    nc = bacc.Bacc("TRN2", target_bir_lowering=False, debug=False,
                   num_devices=N_CORES)

    xq_d = [nc.dram_tensor(f"xq{ch}", [P, DCH, QC], BF16,
                           kind="ExternalInput") for ch in range(NQC)]
    wqT = nc.dram_tensor("wqT", [P, 2, DCH, P], BF16, kind="ExternalInput")
    wkT = nc.dram_tensor("wkT", [P, 2, DCH, P], BF16, kind="ExternalInput")
    wvT = nc.dram_tensor("wvT", [P, DCH, FL], BF16, kind="ExternalInput")
    woT = nc.dram_tensor("woT", [P, 2, D], BF16, kind="ExternalInput")
    bqk = nc.dram_tensor("bqk", [P, 4], F32, kind="ExternalInput")
    out = nc.dram_tensor("out", [S, D], BF16, kind="ExternalOutput")

    with tile.TileContext(nc) as tc:
        _build(nc, tc, xq_d, wqT, wkT, wvT, woT, bqk, out)
    nc.compile()
    return nc


def _build(nc, tc, xq_d, wqT, wkT, wvT, woT, bqk, out):
    from contextlib import ExitStack

    ctx = ExitStack()
    consts = ctx.enter_context(tc.tile_pool(name="consts", bufs=1))
    bqk_sb = consts.tile([P, 4], F32, name="bqk_sb")
    ones_f = consts.tile([1, HD], F32, name="ones_f")
    nc.vector.memset(ones_f[:], 1.0)
    wup = consts.tile([P, P], BF16, name="wup")
    nc.vector.memset(wup[:], 0.0)

    # ---- batched input DMA (host pre-layouts everything [p, d, f]) ----
    xt_pool = ctx.enter_context(tc.tile_pool(name="xt", bufs=1))
    xq = [xt_pool.tile([P, DCH, QC], BF16, name=f"xq{ch}")
          for ch in range(NQC)]
    w_pool = ctx.enter_context(tc.tile_pool(name="w", bufs=1))
    wk_all = w_pool.tile([P, 2, DCH, P], BF16, name="wk_all")
    wq_all = w_pool.tile([P, 2, DCH, P], BF16, name="wq_all")
    wv_all = w_pool.tile([P, DCH, FL], BF16, name="wv_all")
    wo_all = w_pool.tile([P, 2, D], BF16, name="wo_all")

    # 3 parallel DMA queues, d-third granularity, in consumption order:
    # wk + x0 (first-matmul critical path), wq, wv, then the rest.
    qs = [nc.sync, nc.gpsimd, nc.scalar]
    parts = [slice(0, 3), slice(3, 6), slice(6, 8)]
    nc.sync.dma_start(bqk_sb[:], bqk.ap())
    for q, dq in zip(qs, parts):
        q.dma_start(wk_all[:, 0, dq, :], wkT.ap()[:, 0, dq, :])
    for q, dq in zip(qs, parts):
        q.dma_start(xq[0][:, dq, :], xq_d[0].ap()[:, dq, :])
    for q, dq in zip(qs, parts):
        q.dma_start(wq_all[:, 0, dq, :], wqT.ap()[:, 0, dq, :])
    for q, dq in zip(qs, parts):
        q.dma_start(wv_all[:, dq, :], wvT.ap()[:, dq, :])
    for ch in range(1, 4):
        for q, dq in zip(qs, parts):
            q.dma_start(xq[ch][:, dq, :], xq_d[ch].ap()[:, dq, :])
    for q, dq in zip(qs, parts):
        q.dma_start(wk_all[:, 1, dq, :], wkT.ap()[:, 1, dq, :])
    for q, dq in zip(qs, parts):
        q.dma_start(wq_all[:, 1, dq, :], wqT.ap()[:, 1, dq, :])
    nc.sync.dma_start(wo_all[:], woT.ap())

    # ---- persistent compute tiles ----
    kv_pool = ctx.enter_context(tc.tile_pool(name="kv", bufs=1))
    kt = [kv_pool.tile([P, S], BF16, name=f"kt{t}") for t in range(2)]
    qpair = [kv_pool.tile([P, S], BF16, name=f"qp{t}") for t in range(2)]
    vt = [kv_pool.tile([P, VW], BF16, name=f"vt{g}") for g in range(NKK)]
    for g in range(NKK):
        v3 = vt[g][:, 0:HL * (HD + 1)].rearrange("p (h c) -> p h c", c=HD + 1)
        nc.vector.memset(v3[:, :, HD:HD + 1], 1.0)
        nc.vector.memset(vt[g][:, HL * (HD + 1):VW], 0.0)
    attn_sb = [kv_pool.tile([P, S], BF16, name=f"asb{t}") for t in range(2)]

    small_pool = ctx.enter_context(tc.tile_pool(name="small", bufs=4))
    osb_pool = ctx.enter_context(tc.tile_pool(name="osb", bufs=4))

    # PSUM: sc 2x[128,1024]=4 banks, at 2x[65,512]=2, op 2x[128,512]=2.
    with tc.tile_pool(name="sc_ps", bufs=2, space="PSUM") as sc_ps, \
         tc.tile_pool(name="at_ps", bufs=2, space="PSUM") as at_ps, \
         tc.tile_pool(name="op_ps", bufs=2, space="PSUM") as op_ps, \
         tc.tile_pool(name="e_sb", bufs=10) as e_pool:

        # HAM warmup: ~8us of zero matmuls while the input DMAs land, so
        # the PE clock is at 8/8 when real work starts. Results unread.
        wps = sc_ps.tile([P, 2 * QC], F32, name="wps", tag="sc")
        for i in range(56):
            nc.tensor.matmul(wps[:, 0:P], wup[:], wup[:],
                             start=(i == 0), stop=(i == 55))


        def proj_half(which, t, j, act=False):
            """K or Q projection for tile t, columns 512j:512(j+1)."""
            ps = op_ps.tile([P, QC], F32, name=f"ps{which}{t}_{j}", tag="op")
            w = wk_all if which == "k" else wq_all
            co = QC * j
            for d in range(DCH):
                nc.tensor.matmul(ps[:],
                                 w[:, t, d, :],
                                 xq[j][:, d, :],
                                 start=(d == 0), stop=(d == DCH - 1))
            dst = kt[t] if which == "k" else qpair[t]
            bcol = (2 + t) if which == "k" else t
            if act:
                nc.scalar.activation(dst[:, co:co + QC], ps[:], AF.Identity,
                                     bias=bqk_sb[:, bcol:bcol + 1])
            else:
                nc.vector.tensor_scalar_add(dst[:, co:co + QC], ps[:],
                                            bqk_sb[:, bcol:bcol + 1])

        def vproj_pair(pr):
            ps = op_ps.tile([P, 2 * FL], F32, name=f"psv{pr}", tag="op")
            for half in range(2):
                st = 2 * pr + half
                for d in range(DCH):
                    nc.tensor.matmul(ps[:, FL * half:FL * (half + 1)],
                                     xq[st // 4][:, d,
                                                 P * (st % 4):
                                                 P * (st % 4 + 1)],
                                     wv_all[:, d, :],
                                     start=(d == 0), stop=(d == DCH - 1))
            for half in range(2):
                st = 2 * pr + half
                v3 = vt[st][:, 0:HL * (HD + 1)].rearrange(
                    "p (h c) -> p h c", c=HD + 1)
                nc.vector.tensor_copy(
                    v3[:, :, 0:HD],
                    ps[:, FL * half:FL * (half + 1)].rearrange(
                        "p (h dd) -> p h dd", dd=HD))

        op_live = {}

        def run_op_half(st, phase, act_evac=False):
            if phase == 0:
                ops = [op_ps.tile([P, QC], F32, name=f"op{st}_{eb}", tag="op")
                       for eb in range(2)]
                op_live[st] = ops
                for eb in range(2):
                    nc.tensor.matmul(ops[eb][:],
                                     attn_sb[0][:, P * st:P * (st + 1)],
                                     wo_all[:, 0, QC * eb:QC * (eb + 1)],
                                     start=True, stop=False)
                return
            ops = op_live.pop(st)
            for eb in range(2):
                nc.tensor.matmul(ops[eb][:],
                                 attn_sb[1][:, P * st:P * (st + 1)],
                                 wo_all[:, 1, QC * eb:QC * (eb + 1)],
                                 start=False, stop=True)
            osb = osb_pool.tile([P, D], BF16, name=f"osb{st}", tag="osb")
            if act_evac:
                nc.scalar.activation(osb[:, 0:QC], ops[0][:], AF.Identity)
                nc.sync.dma_start(out.ap()[P * st:P * (st + 1), 0:QC],
                                  osb[:, 0:QC])
                nc.vector.tensor_copy(osb[:, QC:2 * QC], ops[1][:])
                nc.sync.dma_start(out.ap()[P * st:P * (st + 1), QC:2 * QC],
                                  osb[:, QC:2 * QC])
            else:
                nc.vector.tensor_copy(osb[:, 0:QC], ops[0][:])
                nc.vector.tensor_copy(osb[:, QC:2 * QC], ops[1][:])
                nc.sync.dma_start(out.ap()[P * st:P * (st + 1), :], osb[:])

        def normalize_evac(qc, t, par, at, fine=False):
            """One [65,512] copy evacuates the whole at-psum tile (frees
            the ring slot in a single op)."""
            atz = small_pool.tile([HD + 1, QC], F32,
                                  name=f"atz{qc}_{t}_{par}", tag="atz")
            if fine and par == 1:
                nc.scalar.activation(atz[:], at[:], AF.Identity)
            else:
                nc.vector.tensor_copy(atz[:], at[:])
            return atz

        def normalize(qc, t, par, atz, fine=False):
            """attn_sb[t][64par:64par+64, qc] = atz[0:64]/atz[64]."""
            zr = small_pool.tile([1, QC], F32, name=f"zr{qc}_{t}_{par}",
                                 tag="zr")
            if fine and par == 0:
                nc.scalar.activation(zr[:], atz[HD:HD + 1, :], AF.Identity)
            else:
                nc.vector.tensor_copy(zr[:], atz[HD:HD + 1, :])
            rz = small_pool.tile([1, QC], F32, name=f"rz{qc}_{t}_{par}",
                                 tag="rz")
            nc.vector.reciprocal_approx_fast(rz[:], zr[:])
            if fine:
                # PE broadcast via ones (PE is free at the tail) + chunked
                # multiplies so the last out-proj can start per 128 cols.
                rzp = op_ps.tile([HD, QC], F32, name=f"rzp{qc}_{t}_{par}",
                                 tag="op")
                nc.tensor.matmul(rzp[:], ones_f[:], rz[:],
                                 start=True, stop=True)
                rzsb = small_pool.tile([HD, QC], F32,
                                       name=f"rzsb{qc}_{t}_{par}", tag="rzb")
                nc.vector.tensor_copy(rzsb[:], rzp[:])
                for ch in range(4):
                    cs = slice(P * ch, P * (ch + 1))
                    nc.vector.tensor_mul(
                        attn_sb[t][HD * par:HD * (par + 1),
                                   QC * qc + P * ch:QC * qc + P * (ch + 1)],
                        atz[0:HD, cs], rzsb[:, cs])
            else:
                rzb = small_pool.tile([HD, QC], F32,
                                      name=f"rzb{qc}_{t}_{par}", tag="rzb")
                nc.gpsimd.partition_broadcast(rzb[:], rz[:])
                mul_queue.append((cur_slot[0], qc, t, par, atz, rzb))

        # ---- slot schedule ----
        # slot s = qc*32 + t*16 + kk ; each slot: 2 concurrent score MMs
        # (row-tiled par 0/1) -> exp[128,1024] -> (pend) 2 PV MMs.
        slots = [(qc, t, kk) for qc in range(NQC) for t in range(2)
                 for kk in range(NKK)]

        # proj work queue: (deadline_slot, kind, (t, j) or g)
        projq = []
        for t in range(2):
            for j in range(NQC):
                if (t, j) != (0, 0):
                    projq.append((t * 16 + 4 * j, "k", (t, j)))
                    projq.append((j * 32 + t * 16, "q", (t, j)))
        for g in range(1, DCH):
            projq.append((2 * g, "v", g))
        projq.sort(key=lambda x: x[0])

        mul_queue = []
        rq_queue = []
        cur_slot = [0]

        def emit_muls(s):
            # age-gate: only emit muls whose gpsimd broadcast has had ~2
            # slots to finish, so they never stall the strict-FIFO VEC
            # queue (which would delay psum-ring-freeing evacs).
            while mul_queue and mul_queue[0][0] <= s - 2:
                _, qc, t, par, atz, rzb = mul_queue.pop(0)
                nc.vector.tensor_mul(
                    attn_sb[t][HD * par:HD * (par + 1),
                               QC * qc:QC * (qc + 1)],
                    atz[0:HD, :], rzb[:])

        pend = []            # (qc, t, kk, par, slot, e_tile, col_off)
        at_tiles = {}
        vready = {}
        op_queue = []
        norm_queue = []

        def flush_norms(s):
            # stage 2 (slot s-1's evacs): recip + broadcast, queue the mul
            while rq_queue and rq_queue[0][0] <= s - 1:
                _, qc, t, par, atz, fine = rq_queue.pop(0)
                normalize(qc, t, par, atz, fine=fine)
                if par == 1 and t == 1:
                    op_queue.extend((st, ph, s + 4)
                                    for st in range(4 * qc, 4 * qc + 4)
                                    for ph in range(2))
            # stage 1 (new kk=15 drains): evacuate the at-psum now
            for qc, t, par, at, fine in norm_queue:
                atz = normalize_evac(qc, t, par, at, fine=fine)
                rq_queue.append((s, qc, t, par, atz, fine))
            del norm_queue[:]

        def drain_one(s, force=False):
            if not pend:
                return False
            qc, t, kk, par, g, et, off = pend[0]
            if not force:
                if g >= s - 2:
                    return False
                if vready.get(kk // 2, 10 ** 9) > s - 4:
                    return False
            pend.pop(0)
            key = (qc, t, par)
            if key not in at_tiles:
                at_tiles[key] = at_ps.tile([HD + 1, QC], F32,
                                           name=f"at{qc}_{t}_{par}", tag="at")
            h = 2 * t + par
            nc.tensor.matmul(at_tiles[key][:], vt[kk][:, 65 * h:65 * h + 65],
                             et[:, off:off + QC],
                             start=(kk == 0), stop=(kk == NKK - 1))
            if kk == NKK - 1:
                last = (qc == NQC - 1 and t == 1)
                norm_queue.append((qc, t, par, at_tiles.pop(key), last))
            return True

        # preamble: minimum before the first score slot
        proj_half("k", 0, 0, act=True)
        proj_half("q", 0, 0, act=True)
        projq.insert(0, (2, "v", 0))

        for s, (qc, t, kk) in enumerate(slots):
            cur_slot[0] = s
            sc = sc_ps.tile([P, 2 * QC], F32, name=f"sc{s}", tag="sc")
            for par in range(2):
                nc.tensor.matmul(sc[:, QC * par:QC * (par + 1)],
                                 kt[t][HD * par:HD * (par + 1),
                                       P * kk:P * (kk + 1)],
                                 qpair[t][HD * par:HD * (par + 1),
                                          QC * qc:QC * (qc + 1)],
                                 start=True, stop=True)
            e = e_pool.tile([P, 2 * QC], BF16, name=f"e{s}", tag="e")
            nc.scalar.activation(e[:], sc[:], AF.Exp)
            for par in range(2):
                pend.append((qc, t, kk, par, s, e, QC * par))
            while projq and projq[0][0] <= s + 3 and not op_live:
                _, kind, arg = projq.pop(0)
                if kind == "v":
                    vready[arg] = s
                    vproj_pair(arg)
                else:
                    proj_half(kind, *arg)
            drained = 0
            while drained < 3 and drain_one(s):
                drained += 1
            emit_muls(s)
            flush_norms(s)
            if s % 2 == 1 and op_queue and op_queue[0][2] <= s:
                st, ph, _ = op_queue.pop(0)
                run_op_half(st, ph)
        early_ph0 = (4 * (NQC - 1), 4 * (NQC - 1) + 1)
        for st in early_ph0:
            pstile = sc_ps.tile([P, 2 * QC], F32, name=f"tailop{st}",
                                tag="sc")
            ops = [pstile[:, 0:QC], pstile[:, QC:2 * QC]]
            op_live[st] = ops
            for eb in range(2):
                nc.tensor.matmul(ops[eb],
                                 attn_sb[0][:, P * st:P * (st + 1)],
                                 wo_all[:, 0, QC * eb:QC * (eb + 1)],
                                 start=True, stop=False)
        while pend:
            drain_one(10 ** 9, force=True)
        cur_slot[0] = 10 ** 9
        flush_norms(10 ** 9)
        flush_norms(10 ** 9 + 2)
        emit_muls(10 ** 9 + 10)
        nq = len(op_queue)
        done = 0
        for st, ph, _ in op_queue:
            if ph == 0 and st in early_ph0:
                continue
            done += 1
            run_op_half(st, ph, act_evac=(done >= nq - 6))
        del op_queue[:]

    ctx.close()


_CACHE = {}


def _get_program():
    if "nc" not in _CACHE:
        _CACHE["nc"] = build_program()
    return _CACHE["nc"]


def prep_inputs(input_tensor, qkv_weight, qkv_bias, out_weight, out_bias):
    """Host-side shard + transpose + cast. Returns in_maps for 8 cores."""
    x = np.asarray(input_tensor, np.float32)
    wqkv = np.asarray(qkv_weight, np.float32).copy()
    bqkv = np.asarray(qkv_bias, np.float32).copy()
    wout = np.asarray(out_weight, np.float32)
    scale = 1.0 / np.sqrt(np.float32(HD))
    wqkv[:D] *= scale
    bqkv[:D] *= scale
    bf = ml_dtypes.bfloat16

    def pdf(wT):
        # [D, F] -> [p, d, F] with D = d*128 + p
        return np.ascontiguousarray(
            wT.reshape(DCH, P, -1).transpose(1, 0, 2)).astype(bf)

    woutT = np.ascontiguousarray(wout.T)
    # x[b].T is [D, S]; slabs [p, d, 512] per column block
    xqb = []
    for b in range(B):
        xt = x[b].T.reshape(DCH, P, S).transpose(1, 0, 2)
        xqb.append([np.ascontiguousarray(xt[:, :, QC * ch:QC * (ch + 1)])
                    .astype(bf) for ch in range(NQC)])
    in_maps = []
    for c in range(N_CORES):
        b, g = c // G, c % G
        lo = FL * g
        def pdf_t(wT):
            # [D, F=256] -> [p, t, d, 128]: F = t*128 + fp, D = d*128 + p
            a = wT.reshape(DCH, P, 2, P)          # [d, p, t, fp]
            return np.ascontiguousarray(
                a.transpose(1, 2, 0, 3)).astype(bf)

        wqT = pdf_t(wqkv[lo:lo + FL, :].T)
        wkT = pdf_t(wqkv[D + lo:D + lo + FL, :].T)
        wvT = pdf(wqkv[2 * D + lo:2 * D + lo + FL, :].T)
        woTg = np.ascontiguousarray(
            woutT[lo:lo + FL, :].reshape(2, P, D).transpose(1, 0, 2)
        ).astype(bf)
        bq = bqkv[lo:lo + FL].reshape(2, P).T
        bk = bqkv[D + lo:D + lo + FL].reshape(2, P).T
        bqk = np.ascontiguousarray(
            np.concatenate([bq, bk], 1)).astype(np.float32)
        im = {"wqT": wqT, "wkT": wkT, "wvT": wvT,
              "woT": woTg, "bqk": bqk}
        for ch in range(NQC):
            im[f"xq{ch}"] = xqb[b][ch]
        in_maps.append(im)
    return in_maps


def assemble(outs, qkv_bias, out_weight, out_bias):
    """Sum the per-core partials and add the (V-bias-folded) output bias."""
    bqkv = np.asarray(qkv_bias, np.float32)
    wout = np.asarray(out_weight, np.float32)
    bout_eff = np.asarray(out_bias, np.float32) + wout @ bqkv[2 * D:]
    full = np.empty((B, S, D), np.float32)
    for b in range(B):
        acc = bout_eff[None, :].astype(np.float32).repeat(S, 0)
        for g in range(G):
            acc += np.asarray(outs[b * G + g], np.float32)
        full[b] = acc
    return full


def kernel(input_tensor, qkv_weight, qkv_bias, out_weight, out_bias,
           **run_kwargs):
    nc = _get_program()
    in_maps = prep_inputs(input_tensor, qkv_weight, qkv_bias, out_weight,
                          out_bias)
    res = run_bass_kernel_spmd(nc, in_maps, core_ids=list(range(N_CORES)),
                               **run_kwargs)
    full = assemble([res.results[c]["out"] for c in range(N_CORES)],
                    qkv_bias, out_weight, out_bias)
    if run_kwargs:
        kernel.last_results = res
    return full
